# revision 8
# baseline (speedup 1.0000x reference)
"""Trainium2 Bass kernel for CropAndResize (bilinear, TF semantics).

Design (8 NeuronCores, image batch-sharded; boxes routed by box_ind):

Each core handles one image of the batch and the boxes pointing at it
(padded to the max per-core box count so one SPMD program serves all
cores). Partition p carries channel pair (p, p+128).

ap_gather on GPSIMD costs ~32 ns per INDEX (async Q7 ucode; d nearly
free), so the kernel uses ONE index per output point with d=8: each
index fetches a 16-byte block holding all four bilinear neighbors for
both channels of its partition:

    W8[p, k, :] = (A[k], A[k+1], A[k+W], A[k+W+1],
                   B[k], B[k+1], B[k+W], B[k+W+1])

W8 is built per 20-row phase (plus 1 halo row) from plain f16 window
DMA loads via 8 strided tensor-copies whose APs have a [1,2] inner dim
(two f16 per block-slot) -> DVE 2x mode; the host pre-casts the image
to f16 so no dtype conversion happens on device. Two W8 ring slots
(8 phases, ring of 2) let a phase's table build overlap the previous
phases' gathers; the gather in_ap spans both slots so calls may
straddle one phase boundary.

Points are sorted by the phase of their top row into a single call
stream padded with prefix-cum-max semantics across cores (much less
padding than per-phase max). Ring reload points E0..E5 are call
indices baked into the SPMD program. Per 768-point call: one
ap_gather, two f16 MULs by host-premultiplied bilinear weights (zeroed
for out-of-range taps, TF extrapolation=0), two pairwise ADDs, and a
store DMA every 2 calls. The host inverse-permutes the output.
"""

import numpy as np

import concourse.bass as bass
import concourse.bacc as bacc
import concourse.tile as tile
from concourse import mybir
from concourse.bass_utils import run_bass_kernel_spmd

B, C, H, W = 8, 256, 160, 160
CH, CW = 14, 14
HW = H * W
N_CORES = 8
NPH = 8          # row phases
RPP = 20         # rows per phase
NEB = RPP * W    # W8 blocks per window (3200)
WINE = (RPP + 1) * W + 2     # f16 window elements per half (3362)
IMGP = HW + W + 4            # padded image length per channel
NI = 768         # points per gather call
NPC = NI // 16   # idx columns per call in the wrapped layout

F32 = mybir.dt.float32
F16 = mybir.dt.float16
I16 = mybir.dt.int16

_PROGRAM_CACHE = {}


def _ap(base, extra_offset, pattern):
    return bass.AP(base.tensor, base.offset + extra_offset, pattern)


def build_program(key):
    """key: (total_calls, E0..E5) — E[p] is the call index at which the
    ring reload for window p+2 is issued (all calls before E[p] contain
    only points of phases <= p+1)."""
    total_calls = key[0]
    E = key[1:]
    nstores = (total_calls + 1) // 2
    nc = bacc.Bacc("TRN2", target_bir_lowering=False, debug=False)

    img_d = nc.dram_tensor("img", [256 * IMGP], F16, kind="ExternalInput")
    idx_d = nc.dram_tensor("idxs", [128, total_calls * NPC], I16,
                           kind="ExternalInput")
    w4_d = nc.dram_tensor("w4", [total_calls, NI * 4], F16, kind="ExternalInput")
    out_d = nc.dram_tensor("out", [nstores, 128, 2 * NI * 2], F16,
                           kind="ExternalOutput")

    A = mybir.AluOpType
    ADD, MUL = A.add, A.mult

    with tile.TileContext(nc) as tc:
        with (
            tc.tile_pool(name="big", bufs=1) as bigp,
            tc.tile_pool(name="win", bufs=4) as winp,
            tc.tile_pool(name="gtp", bufs=2) as gtp,
            tc.tile_pool(name="wp", bufs=2) as wp,
            tc.tile_pool(name="o1p", bufs=2) as o1p,
            tc.tile_pool(name="stg", bufs=2) as stgp,
        ):
            idxs = bigp.tile([128, total_calls * NPC], I16, tag="idxs")
            nc.sync.dma_start(idxs[:], idx_d[:])
            # W8 ring: 2 slots x NEB blocks x 8 f16
            G = bigp.tile([128, 2, NEB, 8], F16, tag="G")
            in_ap = _ap(G[:], 0, [G[:].ap[0], [8, 2 * NEB], [1, 8]])

            def load_windows(ph):
                """DMA the two channel-half f16 windows for phase ph."""
                tiles = []
                for half in (0, 1):
                    wt = winp.tile([128, WINE], F16, tag="win")
                    off = half * 128 * IMGP + ph * RPP * W
                    nc.sync.dma_start(
                        wt[:], _ap(img_d[:], off, [[IMGP, 128], [1, WINE]])
                    )
                    tiles.append(wt)
                return tiles

            def build_w8(ph, wins):
                """8 strided copies window -> W8 ring slot (DVE 2x + Scalar)."""
                slot = ph % 2
                for half in (0, 1):
                    wt = wins[half]
                    for tb in (0, 1):
                        s0 = half * 4 + tb * 2
                        dst = _ap(G[:, slot], s0, [G[:].ap[0], [8, NEB], [1, 2]])
                        src = _ap(wt[:], tb * W, [wt[:].ap[0], [1, NEB], [1, 2]])
                        eng = nc.vector if (half == tb) else nc.scalar
                        if eng is nc.vector:
                            eng.tensor_copy(out=dst, in_=src)
                        else:
                            eng.copy(dst, src)

            wins = {ph: load_windows(ph) for ph in (0, 1, 2)}
            build_w8(0, wins.pop(0))
            build_w8(1, wins.pop(1))

            reload_at = {}
            for p in range(NPH - 2):
                if E[p] < total_calls:
                    reload_at.setdefault(E[p], []).append(p + 2)

            stgt = None
            for ci in range(total_calls):
                for wph in reload_at.get(ci, ()):
                    build_w8(wph, wins.pop(wph))
                    if wph + 1 < NPH:
                        wins[wph + 1] = load_windows(wph + 1)
                gt = gtp.tile([128, NI, 8], F16, tag="gt")
                nc.gpsimd.ap_gather(
                    gt[:], in_ap,
                    idxs[:, ci * NPC:(ci + 1) * NPC],
                    channels=128, num_elems=2 * NEB, d=8, num_idxs=NI,
                )
                w4 = wp.tile([128, NI * 4], F16, tag="w4")
                nc.sync.dma_start(
                    w4[:], _ap(w4_d[:], ci * NI * 4, [[0, 128], [1, NI * 4]])
                )
                # gt layout per point q: [half, tb, e] (8 f16)
                w_ap = _ap(w4[:], 0, [w4[:].ap[0], [4, NI], [1, 4]])
                for half in (0, 1):
                    g_ap = _ap(gt[:], half * 4,
                               [gt[:].ap[0], [8, NI], [1, 4]])
                    nc.vector.tensor_tensor(
                        out=g_ap, in0=g_ap, in1=w_ap, op=MUL
                    )
                # sum tb pairs: o1[q, half, e] = gt[q,half,0,e] + gt[q,half,1,e]
                o1 = o1p.tile([128, NI * 4], F16, tag="o1")
                a0 = _ap(gt[:], 0, [gt[:].ap[0], [8, NI], [4, 2], [1, 2]])
                a1 = _ap(gt[:], 2, [gt[:].ap[0], [8, NI], [4, 2], [1, 2]])
                oo = _ap(o1[:], 0, [o1[:].ap[0], [4, NI], [2, 2], [1, 2]])
                nc.vector.tensor_tensor(out=oo, in0=a0, in1=a1, op=ADD)
                # sum e pairs: out[q*2+half] = o1[q,half,0] + o1[q,half,1]
                if ci % 2 == 0:
                    stgt = stgp.tile([128, 2, NI * 2], F16, tag="stg")
                so = _ap(stgt[:], (ci % 2) * NI * 2,
                         [stgt[:].ap[0], [1, NI * 2]])
                e0 = _ap(o1[:], 0, [o1[:].ap[0], [2, NI * 2]])
                e1 = _ap(o1[:], 1, [o1[:].ap[0], [2, NI * 2]])
                nc.vector.tensor_tensor(out=so, in0=e0, in1=e1, op=ADD)
                if ci % 2 == 1:
                    nc.sync.dma_start(
                        out_d[ci // 2],
                        stgt[:].rearrange("p a b -> p (a b)"),
                    )
            if total_calls % 2 == 1:  # flush odd final store
                nc.sync.dma_start(
                    out_d[total_calls // 2],
                    stgt[:].rearrange("p a b -> p (a b)"),
                )

    nc.compile()
    return nc


def _host_points(bk):
    """Per-point phase, W8 block index (within window), and weights."""
    f = np.float32
    iota = np.arange(CH, dtype=f)
    y1, x1, y2, x2 = bk[:, 0], bk[:, 1], bk[:, 2], bk[:, 3]

    def axis(lo, hi):
        scale = (hi - lo) * f(H - 1) / f(CH - 1)
        inv = lo[:, None] * f(H - 1) + iota[None, :] * scale[:, None]
        valid = (inv >= f(0)) & (inv <= f(H - 1))
        fl = np.floor(inv)
        frac = (inv - fl).astype(f)
        lo_i = np.clip(fl, 0, H - 1).astype(np.int64)
        return valid, frac, lo_i

    vy, yl, ti = axis(y1, y2)
    vx, xl, li = axis(x1, x2)

    valid = vy[:, :, None] & vx[:, None, :]          # [nb,14,14]
    ti3 = ti[:, :, None]
    ph = np.where(valid, np.minimum(ti3 // RPP, NPH - 1), 0)
    k = (ti3 - ph * RPP) * W + li[:, None, :]
    k = np.where(valid, k, 0)
    wy1 = yl[:, :, None]
    wx1 = xl[:, None, :]
    w4p = np.stack(
        [(1 - wy1) * (1 - wx1), (1 - wy1) * wx1, wy1 * (1 - wx1), wy1 * wx1],
        axis=-1,
    )
    w4p = np.where(valid[..., None], w4p, f(0))
    return (
        ph.reshape(-1),
        k.reshape(-1).astype(np.int64),
        w4p.reshape(-1, 4).astype(np.float16),
    )


def _plan(pts):
    """Common prefix-cum-max call plan: (ncalls_total, E0..E5) plus the
    per-core padded per-phase segment lengths."""
    ncore = len(pts)
    counts = np.zeros((ncore, NPH), np.int64)
    for kc, p in enumerate(pts):
        for phs in range(NPH):
            counts[kc, phs] = np.count_nonzero(p[0] == phs)
    # iteratively compute padded positions and reload points
    pos = np.zeros(ncore, np.int64)
    seg = np.zeros((ncore, NPH), np.int64)  # padded length of each segment
    E = []
    for phs in range(NPH):
        if phs >= 2:
            # phase `phs` may not start before E[phs-2]*NI
            floor = E[phs - 2] * NI
            pad = np.maximum(floor - pos, 0)
            seg[:, phs - 1] += pad
            pos += pad
        pos += counts[:, phs]
        seg[:, phs] += counts[:, phs]
        E.append(int(-(-int(pos.max()) // NI)))
    CT = E[-1]
    pad = CT * NI - pos
    seg[:, NPH - 1] += pad
    return CT, tuple(E[:NPH - 2]), seg


def _host_streams(phf, kf, wf, seg_k):
    """Phase-sorted padded idx/weight streams for one core."""
    idx_stream, w_stream, pt_stream = [], [], []
    for p in range(NPH):
        sel = np.nonzero(phf == p)[0]
        pad = int(seg_k[p]) - len(sel)
        assert pad >= 0
        slot = p % 2
        kp = np.concatenate([kf[sel] + slot * NEB,
                             np.full(pad, slot * NEB, np.int64)])
        idx_stream.append(kp.astype(np.int16))
        w_stream.append(np.concatenate([wf[sel], np.zeros((pad, 4), np.float16)]))
        pt_stream.append(np.concatenate([sel, np.full(pad, -1, np.int64)]))
    idx_flat = np.concatenate(idx_stream)          # [CT*NI]
    tc = idx_flat.shape[0] // NI
    w4 = np.concatenate(w_stream).reshape(tc, NI * 4)
    pt_of_stream = np.concatenate(pt_stream)

    # wrap: idx q of call c -> partition q%16, col c*NPC + q//16
    wrap = idx_flat.reshape(tc, NPC, 16).transpose(2, 0, 1).reshape(16, tc * NPC)
    idxs = np.tile(wrap, (8, 1))                   # [128, tc*NPC]
    return idxs, w4, pt_of_stream, phf.shape[0]


def make_in_maps(image, boxes, box_ind):
    image = np.asarray(image)
    boxes = np.asarray(boxes, dtype=np.float32)
    box_ind = np.asarray(box_ind, dtype=np.int32)

    order = np.argsort(box_ind, kind="stable")
    counts = np.bincount(box_ind, minlength=N_CORES)
    starts = np.zeros(N_CORES + 1, np.int64)
    starts[1:] = np.cumsum(counts)
    cap = max(1, int(counts.max()))

    pts = []
    for k in range(N_CORES):
        bk = np.zeros((cap, 4), np.float32)
        sel = order[starts[k]: starts[k + 1]]
        bk[: counts[k]] = boxes[sel]
        pts.append(_host_points(bk))

    CT, E, seg = _plan(pts)
    key = (CT,) + E

    in_maps, metas = [], []
    for k in range(N_CORES):
        img_k = np.zeros((256, IMGP), np.float16)
        img_k[:, :HW] = np.asarray(image[k], np.float16).reshape(256, HW)
        idxs, w4, pt_of_stream, npts = _host_streams(*pts[k], seg[k])
        in_maps.append({"img": img_k.reshape(-1), "idxs": idxs, "w4": w4})
        metas.append((pt_of_stream, key, npts))
    return in_maps, order, counts, starts, cap, metas


def kernel(image, boxes, box_ind):
    in_maps, order, counts, starts, cap, metas = make_in_maps(
        image, boxes, box_ind
    )
    key = metas[0][1]
    nc = _PROGRAM_CACHE.get(key)
    if nc is None:
        nc = build_program(key)
        _PROGRAM_CACHE[key] = nc

    res = run_bass_kernel_spmd(nc, in_maps, core_ids=list(range(N_CORES)))

    n = boxes.shape[0]
    out = np.empty((n, C, CH, CW), np.float32)
    for k in range(N_CORES):
        pt_of_stream, key_k, npts = metas[k]
        sel = order[starts[k]: starts[k + 1]]
        arr = res.results[k]["out"]  # [nstores, 128, 2*NI*2] f16
        tcall = key_k[0]
        arr = arr.reshape(-1, 128, 2, NI, 2)
        # [store, p, sub, q, chbit] -> stream index s = (store*2+sub)*NI + q
        arr = arr.transpose(0, 2, 3, 4, 1).reshape(-1, 2, 128)
        arr = arr.reshape(-1, 256)[: tcall * NI]  # [s, chbit*128+p]
        keep = pt_of_stream >= 0
        ptsv = np.empty((npts, 256), np.float16)
        ptsv[pt_of_stream[keep]] = arr[keep]
        ok = (
            ptsv.reshape(cap, CH, CW, 256)
            .transpose(0, 3, 1, 2)
            .astype(np.float32)[: counts[k]]
        )
        out[sel] = ok
    return out


# revision 9
# speedup vs baseline: 1.0519x; 1.0519x over previous
"""Trainium2 Bass kernel for CropAndResize (bilinear, TF semantics).

Design (8 NeuronCores, image batch-sharded; boxes routed by box_ind):

Each core handles one image of the batch and the boxes pointing at it
(padded to the max per-core box count so one SPMD program serves all
cores). Partition p carries channel pair (p, p+128).

ap_gather on GPSIMD costs ~30 ns per INDEX (async Q7 ucode; d nearly
free), so the kernel uses ONE index per output point with d=8: each
index fetches a 16-byte block holding all four bilinear neighbors for
both channels of its partition:

    W8[p, k, :] = (A[k], A[k+1], A[k+W], A[k+W+1],
                   B[k], B[k+1], B[k+W], B[k+W+1])

W8 is built per 16-row phase (plus 1 halo row) from plain f16 window
DMA loads via 8 strided tensor-copies whose APs have a [1,2] inner dim
(two f16 per block-slot) -> DVE 2x mode; the host pre-casts the image
to f16 so no dtype conversion happens on device. Three W8 ring slots
(10 phases, ring of 3) keep reload bubbles small and let calls
straddle phase boundaries (the gather in_ap spans all three slots).

Points are sorted by the phase of their top row into a single call
stream padded with prefix-cum-max semantics across cores (much less
padding than per-phase max). Ring reload points E0..E6 are call
indices baked into the SPMD program. Per 1152-point call: one
ap_gather, two f16 MULs by host-premultiplied bilinear weights (zeroed
for out-of-range taps, TF extrapolation=0), two pairwise ADDs (first
in-place in gt), and a store DMA per call. The host inverse-permutes
the output.
"""

import numpy as np

import concourse.bass as bass
import concourse.bacc as bacc
import concourse.tile as tile
from concourse import mybir
from concourse.bass_utils import run_bass_kernel_spmd

B, C, H, W = 8, 256, 160, 160
CH, CW = 14, 14
HW = H * W
N_CORES = 8
NPH = 10         # row phases
RPP = 16         # rows per phase
RING = 3         # W8 ring slots
NEB = RPP * W    # W8 blocks per window (2560)
WINE = (RPP + 1) * W + 2     # f16 window elements per half (2722)
IMGP = HW + W + 4            # padded image length per channel
NI = 1152        # points per gather call
NPC = NI // 16   # idx columns per call in the wrapped layout

F32 = mybir.dt.float32
F16 = mybir.dt.float16
I16 = mybir.dt.int16

_PROGRAM_CACHE = {}


def _ap(base, extra_offset, pattern):
    return bass.AP(base.tensor, base.offset + extra_offset, pattern)


def build_program(key):
    """key: (total_calls, E0..E6) — E[p] is the call index at which the
    ring reload for window p+3 is issued (all calls before E[p] contain
    only points of phases <= p+2)."""
    total_calls = key[0]
    E = key[1:]
    nc = bacc.Bacc("TRN2", target_bir_lowering=False, debug=False)

    img_d = nc.dram_tensor("img", [256 * IMGP], F16, kind="ExternalInput")
    idx_d = nc.dram_tensor("idxs", [128, total_calls * NPC], I16,
                           kind="ExternalInput")
    w4_d = nc.dram_tensor("w4", [total_calls, NI * 4], F16, kind="ExternalInput")
    out_d = nc.dram_tensor("out", [total_calls, 128, NI * 2], F16,
                           kind="ExternalOutput")

    A = mybir.AluOpType
    ADD, MUL = A.add, A.mult

    with tile.TileContext(nc) as tc:
        with (
            tc.tile_pool(name="big", bufs=1) as bigp,
            tc.tile_pool(name="win", bufs=3) as winp,
            tc.tile_pool(name="gtp", bufs=2) as gtp,
            tc.tile_pool(name="wp", bufs=2) as wp,
            tc.tile_pool(name="stg", bufs=2) as stgp,
        ):
            idxs = bigp.tile([128, total_calls * NPC], I16, tag="idxs")
            nc.sync.dma_start(idxs[:], idx_d[:])
            # W8 ring: RING slots x NEB blocks x 8 f16
            G = bigp.tile([128, RING, NEB, 8], F16, tag="G")
            in_ap = _ap(G[:], 0, [G[:].ap[0], [8, RING * NEB], [1, 8]])

            def load_windows(ph):
                """DMA the two channel-half f16 windows for phase ph."""
                tiles = []
                for half in (0, 1):
                    wt = winp.tile([128, WINE], F16, tag="win")
                    off = half * 128 * IMGP + ph * RPP * W
                    nc.scalar.dma_start(
                        wt[:], _ap(img_d[:], off, [[IMGP, 128], [1, WINE]])
                    )
                    tiles.append(wt)
                return tiles

            def build_w8(ph, wins):
                """8 strided copies window -> W8 ring slot (DVE 2x + Scalar)."""
                slot = ph % RING
                for half in (0, 1):
                    wt = wins[half]
                    for tb in (0, 1):
                        s0 = half * 4 + tb * 2
                        dst = _ap(G[:, slot], s0, [G[:].ap[0], [8, NEB], [1, 2]])
                        src = _ap(wt[:], tb * W, [wt[:].ap[0], [1, NEB], [1, 2]])
                        eng = nc.vector if (half == tb) else nc.scalar
                        if eng is nc.vector:
                            eng.tensor_copy(out=dst, in_=src)
                        else:
                            eng.copy(dst, src)

            wins = {}
            for ph in range(RING):
                wins[ph] = load_windows(ph)
                build_w8(ph, wins.pop(ph))
            if NPH > RING:
                wins[RING] = load_windows(RING)

            reload_at = {}
            for p in range(NPH - RING):
                if E[p] < total_calls:
                    reload_at.setdefault(E[p], []).append(p + RING)

            for ci in range(total_calls):
                for wph in reload_at.get(ci, ()):
                    build_w8(wph, wins.pop(wph))
                    if wph + 1 < NPH:
                        wins[wph + 1] = load_windows(wph + 1)
                gt = gtp.tile([128, NI, 8], F16, tag="gt")
                nc.gpsimd.ap_gather(
                    gt[:], in_ap,
                    idxs[:, ci * NPC:(ci + 1) * NPC],
                    channels=128, num_elems=RING * NEB, d=8, num_idxs=NI,
                )
                w4 = wp.tile([128, NI * 4], F16, tag="w4")
                nc.sync.dma_start(
                    w4[:], _ap(w4_d[:], ci * NI * 4, [[0, 128], [1, NI * 4]])
                )
                # gt layout per point q: [half, tb, e] (8 f16)
                w_ap = _ap(w4[:], 0, [w4[:].ap[0], [4, NI], [1, 4]])
                for half in (0, 1):
                    g_ap = _ap(gt[:], half * 4,
                               [gt[:].ap[0], [8, NI], [1, 4]])
                    nc.vector.tensor_tensor(
                        out=g_ap, in0=g_ap, in1=w_ap, op=MUL
                    )
                # sum tb pairs in place: gt[q,half,0,e] += gt[q,half,1,e]
                a0 = _ap(gt[:], 0, [gt[:].ap[0], [8, NI], [4, 2], [1, 2]])
                a1 = _ap(gt[:], 2, [gt[:].ap[0], [8, NI], [4, 2], [1, 2]])
                nc.vector.tensor_tensor(out=a0, in0=a0, in1=a1, op=ADD)
                # sum e pairs: stgt[q*2+half] = gt[q,half,0,0] + gt[q,half,0,1]
                stgt = stgp.tile([128, NI * 2], F16, tag="stg")
                so = _ap(stgt[:], 0, [stgt[:].ap[0], [2, NI], [1, 2]])
                e0 = _ap(gt[:], 0, [gt[:].ap[0], [8, NI], [4, 2]])
                e1 = _ap(gt[:], 1, [gt[:].ap[0], [8, NI], [4, 2]])
                nc.vector.tensor_tensor(out=so, in0=e0, in1=e1, op=ADD)
                nc.sync.dma_start(out_d[ci], stgt[:])

    nc.compile()
    return nc


def _host_points(bk):
    """Per-point phase, W8 block index (within window), and weights."""
    f = np.float32
    iota = np.arange(CH, dtype=f)
    y1, x1, y2, x2 = bk[:, 0], bk[:, 1], bk[:, 2], bk[:, 3]

    def axis(lo, hi):
        scale = (hi - lo) * f(H - 1) / f(CH - 1)
        inv = lo[:, None] * f(H - 1) + iota[None, :] * scale[:, None]
        valid = (inv >= f(0)) & (inv <= f(H - 1))
        fl = np.floor(inv)
        frac = (inv - fl).astype(f)
        lo_i = np.clip(fl, 0, H - 1).astype(np.int64)
        return valid, frac, lo_i

    vy, yl, ti = axis(y1, y2)
    vx, xl, li = axis(x1, x2)

    valid = vy[:, :, None] & vx[:, None, :]          # [nb,14,14]
    ti3 = ti[:, :, None]
    ph = np.where(valid, np.minimum(ti3 // RPP, NPH - 1), 0)
    k = (ti3 - ph * RPP) * W + li[:, None, :]
    k = np.where(valid, k, 0)
    wy1 = yl[:, :, None]
    wx1 = xl[:, None, :]
    w4p = np.stack(
        [(1 - wy1) * (1 - wx1), (1 - wy1) * wx1, wy1 * (1 - wx1), wy1 * wx1],
        axis=-1,
    )
    w4p = np.where(valid[..., None], w4p, f(0))
    return (
        ph.reshape(-1),
        k.reshape(-1).astype(np.int64),
        w4p.reshape(-1, 4).astype(np.float16),
    )


def _plan(pts):
    """Common prefix-cum-max call plan: (ncalls_total, E0..E6) plus the
    per-core padded per-phase segment lengths."""
    ncore = len(pts)
    counts = np.zeros((ncore, NPH), np.int64)
    for kc, p in enumerate(pts):
        for phs in range(NPH):
            counts[kc, phs] = np.count_nonzero(p[0] == phs)
    pos = np.zeros(ncore, np.int64)
    seg = np.zeros((ncore, NPH), np.int64)  # padded length of each segment
    E = []
    for phs in range(NPH):
        if phs >= RING:
            # phase `phs` may not start before E[phs-RING]*NI
            floor = E[phs - RING] * NI
            pad = np.maximum(floor - pos, 0)
            seg[:, phs - 1] += pad
            pos += pad
        pos += counts[:, phs]
        seg[:, phs] += counts[:, phs]
        E.append(int(-(-int(pos.max()) // NI)))
    CT = E[-1]
    pad = CT * NI - pos
    seg[:, NPH - 1] += pad
    return CT, tuple(E[:NPH - RING]), seg


def _host_streams(phf, kf, wf, seg_k):
    """Phase-sorted padded idx/weight streams for one core."""
    idx_stream, w_stream, pt_stream = [], [], []
    for p in range(NPH):
        sel = np.nonzero(phf == p)[0]
        pad = int(seg_k[p]) - len(sel)
        assert pad >= 0
        slot = p % RING
        kp = np.concatenate([kf[sel] + slot * NEB,
                             np.full(pad, slot * NEB, np.int64)])
        idx_stream.append(kp.astype(np.int16))
        w_stream.append(np.concatenate([wf[sel], np.zeros((pad, 4), np.float16)]))
        pt_stream.append(np.concatenate([sel, np.full(pad, -1, np.int64)]))
    idx_flat = np.concatenate(idx_stream)          # [CT*NI]
    tc = idx_flat.shape[0] // NI
    w4 = np.concatenate(w_stream).reshape(tc, NI * 4)
    pt_of_stream = np.concatenate(pt_stream)

    # wrap: idx q of call c -> partition q%16, col c*NPC + q//16
    wrap = idx_flat.reshape(tc, NPC, 16).transpose(2, 0, 1).reshape(16, tc * NPC)
    idxs = np.tile(wrap, (8, 1))                   # [128, tc*NPC]
    return idxs, w4, pt_of_stream, phf.shape[0]


def make_in_maps(image, boxes, box_ind):
    image = np.asarray(image)
    boxes = np.asarray(boxes, dtype=np.float32)
    box_ind = np.asarray(box_ind, dtype=np.int32)

    order = np.argsort(box_ind, kind="stable")
    counts = np.bincount(box_ind, minlength=N_CORES)
    starts = np.zeros(N_CORES + 1, np.int64)
    starts[1:] = np.cumsum(counts)
    cap = max(1, int(counts.max()))

    pts = []
    for k in range(N_CORES):
        bk = np.zeros((cap, 4), np.float32)
        sel = order[starts[k]: starts[k + 1]]
        bk[: counts[k]] = boxes[sel]
        pts.append(_host_points(bk))

    CT, E, seg = _plan(pts)
    key = (CT,) + E

    in_maps, metas = [], []
    for k in range(N_CORES):
        img_k = np.zeros((256, IMGP), np.float16)
        img_k[:, :HW] = np.asarray(image[k], np.float16).reshape(256, HW)
        idxs, w4, pt_of_stream, npts = _host_streams(*pts[k], seg[k])
        in_maps.append({"img": img_k.reshape(-1), "idxs": idxs, "w4": w4})
        metas.append((pt_of_stream, key, npts))
    return in_maps, order, counts, starts, cap, metas


def kernel(image, boxes, box_ind):
    in_maps, order, counts, starts, cap, metas = make_in_maps(
        image, boxes, box_ind
    )
    key = metas[0][1]
    nc = _PROGRAM_CACHE.get(key)
    if nc is None:
        nc = build_program(key)
        _PROGRAM_CACHE[key] = nc

    res = run_bass_kernel_spmd(nc, in_maps, core_ids=list(range(N_CORES)))

    n = boxes.shape[0]
    out = np.empty((n, C, CH, CW), np.float32)
    for k in range(N_CORES):
        pt_of_stream, key_k, npts = metas[k]
        sel = order[starts[k]: starts[k + 1]]
        arr = res.results[k]["out"]  # [CT, 128, NI*2] f16
        tcall = key_k[0]
        arr = arr.reshape(tcall, 128, NI, 2)
        # [call, p, q, chbit] -> stream index s = call*NI + q
        arr = arr.transpose(0, 2, 3, 1).reshape(-1, 256)  # [s, chbit*128+p]
        keep = pt_of_stream >= 0
        ptsv = np.empty((npts, 256), np.float16)
        ptsv[pt_of_stream[keep]] = arr[keep]
        ok = (
            ptsv.reshape(cap, CH, CW, 256)
            .transpose(0, 3, 1, 2)
            .astype(np.float32)[: counts[k]]
        )
        out[sel] = ok
    return out
